# revision 26
# baseline (speedup 1.0000x reference)
"""DetectLayer (YOLO-style detection head) Bass kernel for Trainium2, 8 cores.

Data-parallel over batch: 16 images -> 2 per NeuronCore; anchors replicated.

Per (image, anchor) plane of cls_logits [H=128, W=128, C=80], the hot loop
treats each cell's 80 channels as one "page" of a [128, 5120] tile and runs
two full-stream DVE passes:

  pass 1 (native tensor_tensor_scan): segmented running maximum
        state = (reset_j + state) max x_j
    where reset is a constant tile holding -1e9 at page starts - an exact
    per-page reset (data values are never rounded).  The page-end element is
    the cell's channel maximum m.

  pass 2 (one custom DVE op): cumcount = scan(ADD, Src0 < Src1) with
    Src0 = the pass-1 running-max stream and Src1 = the per-page maximum
    broadcast via a stride-0 access pattern.  Within a page,
        #{ j : running_max_j < m }  ==  first argmax index
    exactly (including first-occurrence tie semantics, matching jnp.argmax).
    The scan accumulates monotonically across pages, so per-page counts are
    recovered from adjacent differences of the page-end cumcounts.

This is the minimum DVE work for an exact argmax here: the max must be known
before counting against it, so every element is read exactly twice, and the
free-dim reduce/scan can only run on the vector engine (1 elem/cycle/lane
for fp32).  Everything else (sigmoids, bbox decode, index extraction) is
small and rides the ACT and GPSIMD engines so the DVE stays saturated.

cls_idx is bit-exact vs the jax reference; cls_score uses
sigmoid(max(logits)) == max(sigmoid(logits)) (sigmoid is monotone).
"""

import sys

import numpy as np

try:
    import concourse.bass as bass  # noqa: F401
except ImportError:  # pragma: no cover
    sys.path.insert(0, "/opt/trn_rl_repo")
    import concourse.bass as bass  # noqa: F401

import concourse.bacc as bacc
import concourse.mybir as mybir
import concourse.tile as tile

# ---------------------------------------------------------------------------
# Custom DVE op registration (runtime plugin into concourse's op registry)
# ---------------------------------------------------------------------------
from concourse.dve_ops import (
    OPS,
    CUSTOM_DVE_SPECS,
    DveOp,
    _CUSTOM_DVE_ROW_BASE,
    _SUB_OPCODE_FOR_NAME,
)
from concourse.dve_spec import Spec, Src0, Src1, lower, scan, _has_src1
from concourse.dve_uop import AluOp, DveOpSpec


def _register_op(name, spec, subdim=False):
    if name in _SUB_OPCODE_FOR_NAME:
        return next(o for o in OPS if o.name == name)
    row = _CUSTOM_DVE_ROW_BASE + len(OPS)
    assert row < 0x20
    _SUB_OPCODE_FOR_NAME[name] = row
    shas = {}
    for ver in ("v3", "v4"):
        s = DveOpSpec(
            name=name, opcode=row, uops=lower(spec, ver=ver), rd1_en=_has_src1(spec)
        )
        shas[ver] = s.sha(ver)
    op = DveOp(name, spec, subdim=subdim, uops_sha=shas)
    OPS.append(op)
    CUSTOM_DVE_SPECS[name] = spec
    return op


def _lt_cumcount_ref(in0, in1, s0, s1, imm2):
    P = in0.shape[0]
    ind = (in0.reshape(P, -1) < in1.reshape(P, -1)).astype(np.float32)
    return np.cumsum(ind, axis=1).reshape(in0.shape)


LT_CUMCOUNT = _register_op(
    "LT_CUMCOUNT_ANT",
    Spec(body=scan(AluOp.ADD, Src0 < Src1), reference=_lt_cumcount_ref),
)

# ---------------------------------------------------------------------------
# Problem constants (hardcoded per harness contract)
# ---------------------------------------------------------------------------
B, A, H, W, C = 16, 3, 128, 128, 80
STRIDE = 8
NCORES = 8
BC = B // NCORES  # images per core
WH = W // 2  # planes processed in two half-width chunks
FD = WH * C  # 5120 free elems per partition in the cls pass

F32 = mybir.dt.float32
I32 = mybir.dt.int32

_cache = {}


def _build_module():
    from contextlib import ExitStack

    nc = bacc.Bacc("TRN2", target_bir_lowering=False, debug=False)

    tb_d = nc.dram_tensor("t_bbox", [BC, A, H, W, 4], F32, kind="ExternalInput")
    cf_d = nc.dram_tensor("conf_logits", [BC, A, H, W, 1], F32, kind="ExternalInput")
    cl_d = nc.dram_tensor("cls_logits", [BC, A, H, W, C], F32, kind="ExternalInput")
    an_d = nc.dram_tensor("anchors", [A, 2], F32, kind="ExternalInput")

    pb_d = nc.dram_tensor("p_bbox", [BC, A, H, W, 4], F32, kind="ExternalOutput")
    ci_d = nc.dram_tensor("cls_idx", [BC, A, H, W], I32, kind="ExternalOutput")
    co_d = nc.dram_tensor("confs", [BC, A, H, W], F32, kind="ExternalOutput")

    tb_ap, cf_ap, cl_ap = tb_d.ap(), cf_d.ap(), cl_d.ap()
    an_ap = an_d.ap()
    pb_ap, ci_ap, co_ap = pb_d.ap(), ci_d.ap(), co_d.ap()

    Sig = mybir.ActivationFunctionType.Sigmoid
    Sq = mybir.ActivationFunctionType.Square
    Copy = mybir.ActivationFunctionType.Copy

    with tile.TileContext(nc) as tc, ExitStack() as ctx:
        cpool = ctx.enter_context(tc.tile_pool(name="const", bufs=1))
        xpool = ctx.enter_context(tc.tile_pool(name="x", bufs=3))
        rpool = ctx.enter_context(tc.tile_pool(name="r", bufs=2))
        npool = ctx.enter_context(tc.tile_pool(name="cnt", bufs=2))
        spool = ctx.enter_context(tc.tile_pool(name="small", bufs=3))
        bpool = ctx.enter_context(tc.tile_pool(name="bbox", bufs=2))

        # --- one-time constants ---
        # reset tile for the segmented max scan: -1e9 at each page start
        Rt = cpool.tile([128, FD], F32)
        nc.gpsimd.memset(Rt[:], 0.0)
        Rt3 = Rt[:].rearrange("p (s n) -> p s n", n=C)
        nc.gpsimd.memset(Rt3[:, :, 0:1], -1.0e9)

        # xy bias tile: [h, w, 0] = 8w-4 ; [h, w, 1] = 8h-4
        # (ref: (2*sigmoid - 0.5 + grid) * 8 == 16*sigmoid + (8*grid - 4))
        BXY = cpool.tile([128, W * 2], F32)
        BXY3 = BXY[:].rearrange("p (w t) -> p w t", t=2)
        nc.gpsimd.iota(
            BXY3[:, :, 0], pattern=[[8, W]], base=-4, channel_multiplier=0,
            allow_small_or_imprecise_dtypes=True,
        )
        nc.gpsimd.iota(
            BXY3[:, :, 1], pattern=[[0, W]], base=-4, channel_multiplier=8,
            allow_small_or_imprecise_dtypes=True,
        )

        # anchors*4, broadcast to all partitions: a4[:, 2a+k] = 4*anchors[a,k]
        # (ref: (2*sigmoid)^2 * anchor == sigmoid^2 * 4*anchor)
        a4 = cpool.tile([128, A * 2], F32)
        nc.sync.dma_start(
            a4[:],
            an_ap.rearrange("a k -> (a k)").unsqueeze(0).to_broadcast((128, A * 2)),
        )
        nc.scalar.mul(a4[:], a4[:], 4.0)

        for b in range(BC):
            for a in range(A):
                for w0 in (0, WH):
                    # ---------------- cls: max + argmax ----------------
                    xt = xpool.tile([128, FD], F32, tag="x")
                    xt3 = xt[:].rearrange("p (s n) -> p s n", n=C)
                    nc.sync.dma_start(xt3[:], cl_ap[b, a, :, w0 : w0 + WH, :])

                    rt = rpool.tile([128, FD], F32, tag="r")
                    nc.vector.tensor_tensor_scan(
                        out=rt[:], data0=Rt[:], data1=xt[:], initial=-3.0e38,
                        op0=mybir.AluOpType.add, op1=mybir.AluOpType.max,
                    )
                    rt3 = rt[:].rearrange("p (s n) -> p s n", n=C)
                    m_ap = rt3[:, :, C - 1]  # [128, WH] page maxima (strided)
                    m_b = rt3[:, :, C - 1 : C].to_broadcast((128, WH, C))

                    ct = npool.tile([128, FD], F32, tag="cnt")
                    ct3 = ct[:].rearrange("p (s n) -> p s n", n=C)
                    nc.vector._custom_dve(LT_CUMCOUNT, out=ct3, in0=rt3, in1=m_b)

                    # idx = adjacent differences of page-end cumcounts
                    itf = spool.tile([128, WH], F32, tag="idxf")
                    nc.gpsimd.tensor_copy(itf[:, 0:1], ct3[:, 0:1, C - 1])
                    nc.gpsimd.tensor_sub(
                        itf[:, 1:WH], ct3[:, 1:WH, C - 1], ct3[:, 0 : WH - 1, C - 1]
                    )
                    it = spool.tile([128, WH], I32, tag="idx")
                    nc.gpsimd.tensor_copy(it[:], itf[:])
                    nc.sync.dma_start(ci_ap[b, a, :, w0 : w0 + WH], it[:])

                    # ---------------- confs ----------------
                    sc = spool.tile([128, WH], F32, tag="sc")
                    nc.scalar.activation(sc[:], m_ap, Sig)
                    pc = spool.tile([128, WH], F32, tag="pc")
                    nc.sync.dma_start(pc[:], cf_ap[b, a, :, w0 : w0 + WH, 0])
                    cf = spool.tile([128, WH], F32, tag="cf")
                    nc.scalar.activation(cf[:], pc[:], Sig)
                    nc.gpsimd.tensor_mul(cf[:], cf[:], sc[:])
                    nc.sync.dma_start(co_ap[b, a, :, w0 : w0 + WH], cf[:])

                    # ---------------- bbox ----------------
                    tb = bpool.tile([128, WH * 4], F32, tag="tb")
                    tb3 = tb[:].rearrange("p (w t) -> p w t", t=4)
                    nc.sync.dma_start(tb3[:], tb_ap[b, a, :, w0 : w0 + WH, :])
                    sg = bpool.tile([128, WH * 4], F32, tag="sg")
                    sg3 = sg[:].rearrange("p (w t) -> p w t", t=4)
                    nc.scalar.activation(sg[:], tb[:], Sig)

                    ot = bpool.tile([128, WH * 4], F32, tag="ot")
                    ot3 = ot[:].rearrange("p (w t) -> p w t", t=4)
                    # xy = sig*16 + (8*grid - 4), on gpsimd to keep DVE free
                    nc.gpsimd.tensor_scalar_mul(ot3[:, :, 0:2], sg3[:, :, 0:2], 16.0)
                    nc.gpsimd.tensor_add(
                        ot3[:, :, 0:2], ot3[:, :, 0:2], BXY3[:, w0 : w0 + WH, :]
                    )
                    # wh = sig^2 * (4*anchor)
                    sq = bpool.tile([128, WH * 2], F32, tag="sq")
                    sq3 = sq[:].rearrange("p (w t) -> p w t", t=2)
                    nc.scalar.activation(sq3[:], sg3[:, :, 2:4], Sq)
                    nc.scalar.activation(
                        ot3[:, :, 2], sq3[:, :, 0], Copy,
                        scale=a4[:, 2 * a : 2 * a + 1],
                    )
                    nc.scalar.activation(
                        ot3[:, :, 3], sq3[:, :, 1], Copy,
                        scale=a4[:, 2 * a + 1 : 2 * a + 2],
                    )
                    nc.sync.dma_start(pb_ap[b, a, :, w0 : w0 + WH, :], ot3[:])

    nc.compile()
    return nc


def _get_module():
    if "nc" not in _cache:
        _cache["nc"] = _build_module()
    return _cache["nc"]


def _get_runner():
    """Build (once) a jitted shard_map runner over 8 cores.

    Mirrors bass2jax.run_bass_via_pjrt's multi-core path, minus output-buffer
    donation (every output element is written by the kernel, so the custom
    call's result buffers need no zero-fill) so the jit can be re-invoked
    cheaply with device-resident inputs.
    """
    if "runner" in _cache:
        return _cache["runner"]
    import jax
    from jax.sharding import Mesh, PartitionSpec
    from jax.experimental.shard_map import shard_map
    from concourse import bass2jax

    nc = _get_module()
    bass2jax.install_neuronx_cc_hook()

    partition_name = (
        nc.partition_id_tensor.name if nc.partition_id_tensor else None
    )
    in_names, out_names, out_avals = [], [], []
    for alloc in nc.m.functions[0].allocations:
        if not isinstance(alloc, mybir.MemoryLocationSet):
            continue
        name = alloc.memorylocations[0].name
        if alloc.kind == "ExternalInput":
            if name != partition_name:
                in_names.append(name)
        elif alloc.kind == "ExternalOutput":
            out_names.append(name)
            out_avals.append(
                jax.core.ShapedArray(
                    tuple(alloc.tensor_shape), mybir.dt.np(alloc.dtype)
                )
            )
    n_params = len(in_names)
    all_names = in_names + out_names
    if partition_name is not None:
        all_names = all_names + [partition_name]

    def _body(*args):
        operands = list(args)
        if partition_name is not None:
            operands.append(bass2jax.partition_id_tensor())
        outs = bass2jax._bass_exec_p.bind(
            *operands,
            out_avals=tuple(out_avals),
            in_names=tuple(all_names),
            out_names=tuple(out_names),
            lowering_input_output_aliases=(),
            sim_require_finite=True,
            sim_require_nnan=True,
            nc=nc,
        )
        return tuple(outs)

    devices = jax.devices()[:NCORES]
    mesh = Mesh(np.asarray(devices), ("core",))
    nin = n_params + len(out_names)
    sharded = jax.jit(
        shard_map(
            _body,
            mesh=mesh,
            in_specs=(PartitionSpec("core"),) * nin,
            out_specs=(PartitionSpec("core"),) * len(out_names),
            check_rep=False,
        ),
        keep_unused=True,
    )
    zeros = [
        np.zeros((NCORES * a.shape[0], *a.shape[1:]), a.dtype) for a in out_avals
    ]
    _cache["runner"] = (sharded, in_names, out_names, out_avals, zeros)
    return _cache["runner"]


def _run(t_bbox, conf_logits, cls_logits, anchors):
    sharded, in_names, out_names, out_avals, zeros = _get_runner()
    by_name = {
        "t_bbox": t_bbox,
        "conf_logits": conf_logits,
        "cls_logits": cls_logits,
        # replicate anchors per core; shard_map splits axis 0
        "anchors": np.tile(anchors, (NCORES, 1)),
    }
    ins = [by_name[n] for n in in_names]
    outs = sharded(*ins, *zeros)
    return {name: np.asarray(outs[i]) for i, name in enumerate(out_names)}


def kernel(t_bbox, conf_logits, cls_logits, anchors):
    t_bbox = np.ascontiguousarray(t_bbox, dtype=np.float32)
    conf_logits = np.ascontiguousarray(conf_logits, dtype=np.float32)
    cls_logits = np.ascontiguousarray(cls_logits, dtype=np.float32)
    anchors = np.ascontiguousarray(anchors, dtype=np.float32)

    try:
        res = _run(t_bbox, conf_logits, cls_logits, anchors)
    except Exception:
        # transient device/runtime hiccup: rebuild the jitted runner and retry
        import time as _time

        _cache.pop("runner", None)
        _time.sleep(5)
        res = _run(t_bbox, conf_logits, cls_logits, anchors)
    p_bbox = res["p_bbox"].reshape(B, A * H * W, 4)
    cls_idx = res["cls_idx"].reshape(B, A * H * W)
    confs = res["confs"].reshape(B, A * H * W)
    return p_bbox, cls_idx, confs


# revision 31
# speedup vs baseline: 1.4083x; 1.4083x over previous
"""DetectLayer (YOLO-style detection head) Bass kernel for Trainium2, 8 cores.

Data-parallel over batch: 16 images -> 2 per NeuronCore; anchors replicated.

Per (image, anchor) plane of cls_logits [H=128, W=128, C=80], the hot loop
treats each cell's 80 channels as one "page" of a [128, 5120] tile and runs
two full-stream DVE passes:

  pass 1 (native tensor_tensor_scan): segmented running maximum
        state = (reset_j + state) max x_j
    where reset is a constant tile holding -1e9 at page starts - an exact
    per-page reset (data values are never rounded).  The page-end element is
    the cell's channel maximum m.

  pass 2 (one custom DVE op): cumcount = scan(ADD, Src0 < Src1) with
    Src0 = the pass-1 running-max stream and Src1 = the per-page maximum
    broadcast via a stride-0 access pattern.  Within a page,
        #{ j : running_max_j < m }  ==  first argmax index
    exactly (including first-occurrence tie semantics, matching jnp.argmax).
    The scan accumulates monotonically across pages, so per-page counts are
    recovered from adjacent differences of the page-end cumcounts.

This is the minimum DVE work for an exact argmax here: the max must be known
before counting against it, so every element is read exactly twice, and the
free-dim reduce/scan can only run on the vector engine (1 elem/cycle/lane
for fp32).  Everything else (sigmoids, bbox decode, index extraction) is
small and rides the ACT and GPSIMD engines so the DVE stays saturated.

cls_idx is bit-exact vs the jax reference; cls_score uses
sigmoid(max(logits)) == max(sigmoid(logits)) (sigmoid is monotone).
"""

import sys

import numpy as np

try:
    import concourse.bass as bass  # noqa: F401
except ImportError:  # pragma: no cover
    sys.path.insert(0, "/opt/trn_rl_repo")
    import concourse.bass as bass  # noqa: F401

import concourse.bacc as bacc
import concourse.mybir as mybir
import concourse.tile as tile

# ---------------------------------------------------------------------------
# Custom DVE op registration (runtime plugin into concourse's op registry)
# ---------------------------------------------------------------------------
from concourse.dve_ops import (
    OPS,
    CUSTOM_DVE_SPECS,
    DveOp,
    _CUSTOM_DVE_ROW_BASE,
    _SUB_OPCODE_FOR_NAME,
)
from concourse.dve_spec import Spec, Src0, Src1, lower, scan, _has_src1
from concourse.dve_uop import AluOp, DveOpSpec


def _register_op(name, spec, subdim=False):
    if name in _SUB_OPCODE_FOR_NAME:
        return next(o for o in OPS if o.name == name)
    row = _CUSTOM_DVE_ROW_BASE + len(OPS)
    assert row < 0x20
    _SUB_OPCODE_FOR_NAME[name] = row
    shas = {}
    for ver in ("v3", "v4"):
        s = DveOpSpec(
            name=name, opcode=row, uops=lower(spec, ver=ver), rd1_en=_has_src1(spec)
        )
        shas[ver] = s.sha(ver)
    op = DveOp(name, spec, subdim=subdim, uops_sha=shas)
    OPS.append(op)
    CUSTOM_DVE_SPECS[name] = spec
    return op


def _lt_cumcount_ref(in0, in1, s0, s1, imm2):
    P = in0.shape[0]
    ind = (in0.reshape(P, -1) < in1.reshape(P, -1)).astype(np.float32)
    return np.cumsum(ind, axis=1).reshape(in0.shape)


LT_CUMCOUNT = _register_op(
    "LT_CUMCOUNT_ANT",
    Spec(body=scan(AluOp.ADD, Src0 < Src1), reference=_lt_cumcount_ref),
)

# ---------------------------------------------------------------------------
# Problem constants (hardcoded per harness contract)
# ---------------------------------------------------------------------------
B, A, H, W, C = 16, 3, 128, 128, 80
STRIDE = 8
NCORES = 8
BC = B // NCORES  # images per core
WH = W // 2  # planes processed in two half-width chunks
FD = WH * C  # 5120 free elems per partition in the cls pass

F32 = mybir.dt.float32
I32 = mybir.dt.int32

_cache = {}


def _build_module():
    from contextlib import ExitStack

    nc = bacc.Bacc("TRN2", target_bir_lowering=False, debug=False)

    tb_d = nc.dram_tensor("t_bbox", [BC, A, H, W, 4], F32, kind="ExternalInput")
    cf_d = nc.dram_tensor("conf_logits", [BC, A, H, W, 1], F32, kind="ExternalInput")
    cl_d = nc.dram_tensor("cls_logits", [BC, A, H, W, C], F32, kind="ExternalInput")
    an_d = nc.dram_tensor("anchors", [A, 2], F32, kind="ExternalInput")

    pb_d = nc.dram_tensor("p_bbox", [BC, A, H, W, 4], F32, kind="ExternalOutput")
    ci_d = nc.dram_tensor("cls_idx", [BC, A, H, W], I32, kind="ExternalOutput")
    co_d = nc.dram_tensor("confs", [BC, A, H, W], F32, kind="ExternalOutput")

    tb_ap, cf_ap, cl_ap = tb_d.ap(), cf_d.ap(), cl_d.ap()
    an_ap = an_d.ap()
    pb_ap, ci_ap, co_ap = pb_d.ap(), ci_d.ap(), co_d.ap()

    Sig = mybir.ActivationFunctionType.Sigmoid
    Sq = mybir.ActivationFunctionType.Square
    Copy = mybir.ActivationFunctionType.Copy

    with tile.TileContext(nc) as tc, ExitStack() as ctx:
        cpool = ctx.enter_context(tc.tile_pool(name="const", bufs=1))
        xpool = ctx.enter_context(tc.tile_pool(name="x", bufs=3))
        rpool = ctx.enter_context(tc.tile_pool(name="r", bufs=2))
        npool = ctx.enter_context(tc.tile_pool(name="cnt", bufs=2))
        spool = ctx.enter_context(tc.tile_pool(name="small", bufs=3))
        bpool = ctx.enter_context(tc.tile_pool(name="bbox", bufs=2))

        # --- one-time constants ---
        # reset tile for the segmented max scan: -1e9 at each page start.
        # Zero-fill split across DVE (idle during the first load) and GPSIMD
        # so the first scan isn't gated on a single slow memset.
        Rt = cpool.tile([128, FD], F32)
        nc.vector.memset(Rt[:, : FD // 2], 0.0)
        nc.gpsimd.memset(Rt[:, FD // 2 :], 0.0)
        Rt3 = Rt[:].rearrange("p (s n) -> p s n", n=C)
        nc.gpsimd.memset(Rt3[:, : WH // 2, 0:1], -1.0e9)
        nc.vector.memset(Rt3[:, WH // 2 :, 0:1], -1.0e9)

        # xy bias tile: [h, w, 0] = 8w-4 ; [h, w, 1] = 8h-4
        # (ref: (2*sigmoid - 0.5 + grid) * 8 == 16*sigmoid + (8*grid - 4))
        BXY = cpool.tile([128, W * 2], F32)
        BXY3 = BXY[:].rearrange("p (w t) -> p w t", t=2)
        nc.gpsimd.iota(
            BXY3[:, :, 0], pattern=[[8, W]], base=-4, channel_multiplier=0,
            allow_small_or_imprecise_dtypes=True,
        )
        nc.gpsimd.iota(
            BXY3[:, :, 1], pattern=[[0, W]], base=-4, channel_multiplier=8,
            allow_small_or_imprecise_dtypes=True,
        )

        # anchors*4, broadcast to all partitions: a4[:, 2a+k] = 4*anchors[a,k]
        # (ref: (2*sigmoid)^2 * anchor == sigmoid^2 * 4*anchor)
        a4 = cpool.tile([128, A * 2], F32)
        nc.sync.dma_start(
            a4[:],
            an_ap.rearrange("a k -> (a k)").unsqueeze(0).to_broadcast((128, A * 2)),
        )
        nc.scalar.mul(a4[:], a4[:], 4.0)

        niter = BC * A * 2
        for it_i in range(niter):
            if True:
                b, rest = divmod(it_i, A * 2)
                a, half = divmod(rest, 2)
                w0 = half * WH
                if True:
                    first, last = it_i == 0, it_i == niter - 1
                    # ---------------- cls: max + argmax ----------------
                    xt = xpool.tile([128, FD], F32, tag="x")
                    xt3 = xt[:].rearrange("p (s n) -> p s n", n=C)
                    HQ = WH // 2
                    rt = rpool.tile([128, FD], F32, tag="r")
                    if first:
                        # geometric split: DVE starts after an 1/8 load
                        bounds = (0, WH // 8, WH // 4, WH // 2, WH)
                        for lo, hi in zip(bounds, bounds[1:]):
                            nc.sync.dma_start(
                                xt3[:, lo:hi, :],
                                cl_ap[b, a, :, w0 + lo : w0 + hi, :],
                            )
                        for lo, hi in zip(bounds, bounds[1:]):
                            lo, hi = lo * C, hi * C
                            nc.vector.tensor_tensor_scan(
                                out=rt[:, lo:hi], data0=Rt[:, lo:hi],
                                data1=xt[:, lo:hi], initial=-3.0e38,
                                op0=mybir.AluOpType.add, op1=mybir.AluOpType.max,
                            )
                    else:
                        nc.sync.dma_start(xt3[:], cl_ap[b, a, :, w0 : w0 + WH, :])
                        nc.vector.tensor_tensor_scan(
                            out=rt[:], data0=Rt[:], data1=xt[:], initial=-3.0e38,
                            op0=mybir.AluOpType.add, op1=mybir.AluOpType.max,
                        )
                    rt3 = rt[:].rearrange("p (s n) -> p s n", n=C)
                    m_ap = rt3[:, :, C - 1]  # [128, WH] page maxima (strided)

                    ct = npool.tile([128, FD], F32, tag="cnt")
                    ct3 = ct[:].rearrange("p (s n) -> p s n", n=C)
                    if last:
                        # split the count pass so the kernel tail shrinks
                        for lo, hi in ((0, HQ), (HQ, WH)):
                            nc.vector._custom_dve(
                                LT_CUMCOUNT,
                                out=ct3[:, lo:hi, 0 : C - 1],
                                in0=rt3[:, lo:hi, 0 : C - 1],
                                in1=rt3[:, lo:hi, C - 1 : C].to_broadcast(
                                    (128, HQ, C - 1)
                                ),
                            )
                    else:
                        m_b = rt3[:, :, C - 1 : C].to_broadcast((128, WH, C - 1))
                        nc.vector._custom_dve(
                            LT_CUMCOUNT,
                            out=ct3[:, :, 0 : C - 1],
                            in0=rt3[:, :, 0 : C - 1],
                            in1=m_b,
                        )

                    # idx = adjacent differences of page-end cumcounts
                    # (cumcounts restart at each count-pass start, so diff
                    # within each counted segment)
                    segs = ((0, HQ), (HQ, WH)) if last else ((0, WH),)
                    itf = spool.tile([128, WH], F32, tag="idxf")
                    it = spool.tile([128, WH], I32, tag="idx")
                    # last tile: keep the tail chain on the DVE (no cross-
                    # engine handoff after the final count pass)
                    veng = nc.vector if last else nc.gpsimd
                    for lo, hi in segs:
                        veng.tensor_copy(
                            itf[:, lo : lo + 1], ct3[:, lo : lo + 1, C - 2]
                        )
                        veng.tensor_sub(
                            itf[:, lo + 1 : hi],
                            ct3[:, lo + 1 : hi, C - 2],
                            ct3[:, lo : hi - 1, C - 2],
                        )
                    veng.tensor_copy(it[:], itf[:])
                    nc.sync.dma_start(ci_ap[b, a, :, w0 : w0 + WH], it[:])

                    # ---------------- confs ----------------
                    sc = spool.tile([128, WH], F32, tag="sc")
                    nc.scalar.activation(sc[:], m_ap, Sig)
                    pc = spool.tile([128, WH], F32, tag="pc")
                    nc.sync.dma_start(pc[:], cf_ap[b, a, :, w0 : w0 + WH, 0])
                    cf = spool.tile([128, WH], F32, tag="cf")
                    nc.scalar.activation(cf[:], pc[:], Sig)
                    nc.gpsimd.tensor_mul(cf[:], cf[:], sc[:])
                    nc.sync.dma_start(co_ap[b, a, :, w0 : w0 + WH], cf[:])

                    # ---------------- bbox ----------------
                    tb = bpool.tile([128, WH * 4], F32, tag="tb")
                    tb3 = tb[:].rearrange("p (w t) -> p w t", t=4)
                    nc.sync.dma_start(tb3[:], tb_ap[b, a, :, w0 : w0 + WH, :])
                    sg = bpool.tile([128, WH * 4], F32, tag="sg")
                    sg3 = sg[:].rearrange("p (w t) -> p w t", t=4)
                    nc.scalar.activation(sg[:], tb[:], Sig)

                    ot = bpool.tile([128, WH * 4], F32, tag="ot")
                    ot3 = ot[:].rearrange("p (w t) -> p w t", t=4)
                    # xy = sig*16 + (8*grid - 4), on gpsimd to keep DVE free
                    nc.gpsimd.tensor_scalar_mul(ot3[:, :, 0:2], sg3[:, :, 0:2], 16.0)
                    nc.gpsimd.tensor_add(
                        ot3[:, :, 0:2], ot3[:, :, 0:2], BXY3[:, w0 : w0 + WH, :]
                    )
                    # wh = sig^2 * (4*anchor)
                    sq = bpool.tile([128, WH * 2], F32, tag="sq")
                    sq3 = sq[:].rearrange("p (w t) -> p w t", t=2)
                    nc.scalar.activation(sq3[:], sg3[:, :, 2:4], Sq)
                    nc.scalar.activation(
                        ot3[:, :, 2], sq3[:, :, 0], Copy,
                        scale=a4[:, 2 * a : 2 * a + 1],
                    )
                    nc.scalar.activation(
                        ot3[:, :, 3], sq3[:, :, 1], Copy,
                        scale=a4[:, 2 * a + 1 : 2 * a + 2],
                    )
                    nc.sync.dma_start(pb_ap[b, a, :, w0 : w0 + WH, :], ot3[:])

    nc.compile()
    return nc


def _get_module():
    if "nc" not in _cache:
        _cache["nc"] = _build_module()
    return _cache["nc"]


def _get_runner():
    """Build (once) a jitted shard_map runner over 8 cores.

    Mirrors bass2jax.run_bass_via_pjrt's multi-core path, minus output-buffer
    donation (every output element is written by the kernel, so the custom
    call's result buffers need no zero-fill) so the jit can be re-invoked
    cheaply with device-resident inputs.
    """
    if "runner" in _cache:
        return _cache["runner"]
    import jax
    from jax.sharding import Mesh, PartitionSpec
    from jax.experimental.shard_map import shard_map
    from concourse import bass2jax

    nc = _get_module()
    bass2jax.install_neuronx_cc_hook()

    partition_name = (
        nc.partition_id_tensor.name if nc.partition_id_tensor else None
    )
    in_names, out_names, out_avals = [], [], []
    for alloc in nc.m.functions[0].allocations:
        if not isinstance(alloc, mybir.MemoryLocationSet):
            continue
        name = alloc.memorylocations[0].name
        if alloc.kind == "ExternalInput":
            if name != partition_name:
                in_names.append(name)
        elif alloc.kind == "ExternalOutput":
            out_names.append(name)
            out_avals.append(
                jax.core.ShapedArray(
                    tuple(alloc.tensor_shape), mybir.dt.np(alloc.dtype)
                )
            )
    n_params = len(in_names)
    all_names = in_names + out_names
    if partition_name is not None:
        all_names = all_names + [partition_name]

    def _body(*args):
        operands = list(args)
        if partition_name is not None:
            operands.append(bass2jax.partition_id_tensor())
        outs = bass2jax._bass_exec_p.bind(
            *operands,
            out_avals=tuple(out_avals),
            in_names=tuple(all_names),
            out_names=tuple(out_names),
            lowering_input_output_aliases=(),
            sim_require_finite=True,
            sim_require_nnan=True,
            nc=nc,
        )
        return tuple(outs)

    devices = jax.devices()[:NCORES]
    mesh = Mesh(np.asarray(devices), ("core",))
    nin = n_params + len(out_names)
    sharded = jax.jit(
        shard_map(
            _body,
            mesh=mesh,
            in_specs=(PartitionSpec("core"),) * nin,
            out_specs=(PartitionSpec("core"),) * len(out_names),
            check_rep=False,
        ),
        keep_unused=True,
    )
    zeros = [
        np.zeros((NCORES * a.shape[0], *a.shape[1:]), a.dtype) for a in out_avals
    ]
    _cache["runner"] = (sharded, in_names, out_names, out_avals, zeros)
    return _cache["runner"]


def _run(t_bbox, conf_logits, cls_logits, anchors):
    sharded, in_names, out_names, out_avals, zeros = _get_runner()
    by_name = {
        "t_bbox": t_bbox,
        "conf_logits": conf_logits,
        "cls_logits": cls_logits,
        # replicate anchors per core; shard_map splits axis 0
        "anchors": np.tile(anchors, (NCORES, 1)),
    }
    ins = [by_name[n] for n in in_names]
    outs = sharded(*ins, *zeros)
    return {name: np.asarray(outs[i]) for i, name in enumerate(out_names)}


def kernel(t_bbox, conf_logits, cls_logits, anchors):
    t_bbox = np.ascontiguousarray(t_bbox, dtype=np.float32)
    conf_logits = np.ascontiguousarray(conf_logits, dtype=np.float32)
    cls_logits = np.ascontiguousarray(cls_logits, dtype=np.float32)
    anchors = np.ascontiguousarray(anchors, dtype=np.float32)

    try:
        res = _run(t_bbox, conf_logits, cls_logits, anchors)
    except Exception:
        # transient device/runtime hiccup: rebuild the jitted runner and retry
        import time as _time

        _cache.pop("runner", None)
        _time.sleep(5)
        res = _run(t_bbox, conf_logits, cls_logits, anchors)
    p_bbox = res["p_bbox"].reshape(B, A * H * W, 4)
    cls_idx = res["cls_idx"].reshape(B, A * H * W)
    confs = res["confs"].reshape(B, A * H * W)
    return p_bbox, cls_idx, confs
